# revision 1
# baseline (speedup 1.0000x reference)
"""nn_Decode (CenterNet-style polygon decode) on 8 Trainium2 NeuronCores.

Strategy (data-parallel over batch, instance-routed gather):
  host:   all index math: wh_pred center gather (host-known indices), init_polys,
          bilinear corner indices + weights, instance->core routing by image,
          weight layout transforms, bf16 casts.
  device: per core (c = 2*b + h) = (image b, half h):
          conv3x3(64->256)+ReLU+conv1x1(256->64) on its half-image via im2col
          matmuls (bf16, row-pair K-packing), f written pixel-major bf16 to DRAM,
          pair AllGather -> full-image f, dma_gather (int16 idx, 512B rows of
          4px x 64ch), DVE bilinear combine with ACT/GPSIMD weight replication,
          PE transposes -> poly matmul (K=8320) -> fuse matmul -> off2^T out.
  host:   out[0] = init*4 (exact), out[1] = off2*16 + out[0].
"""
import os
import numpy as np
import ml_dtypes

import concourse.bass as bass
import concourse.mybir as mybir
import concourse.tile as tile
from concourse import bacc
from concourse.bass_utils import run_bass_kernel_spmd
from concourse.masks import make_identity

BF16, F32, I16 = mybir.dt.bfloat16, mybir.dt.float32, mybir.dt.int16
AF = mybir.ActivationFunctionType
bf16 = ml_dtypes.bfloat16

# problem constants (hardcoded per spec)
B, CIN, H, W = 4, 64, 256, 256
C1 = 256                 # conv1 out channels
NPT, NP1, NINST = 128, 129, 2000
INIT_STRIDE, COARSE_STRIDE, DOWN = 10.0, 4.0, 4.0

LANES = 128                       # instances per block
SLOTS = 2 * NP1                   # 258 gather slots per block, yc-major: s = yc*129+pt
SC = 43                           # slots per gather chunk
NCH = SLOTS // SC                 # 6 chunks per block (chunks 0-2: yc=0, 3-5: yc=1)
NIDX = SC * LANES                 # 5504 indices per gather
KPOLY, KPAD = NP1 * 64, 65 * 128  # 8256, 8320
HROWS = 130                       # input halo rows per half
WPAD = W + 2                      # 258, zero-padded row width
FHALF = (H // 2) * W              # 32768 px per half
FROWS = 2 * FHALF + 128           # f_full rows incl. pad
NEX = 8                           # exchange chunks (f_full chunk-interleaved)

_PROG_CACHE = {}
TRACE = False          # test harness sets True to capture NTFF profile
LAST_EXEC_NS = None
LAST_RESULT = None


def _build_program(num_devices, pairs, nblk):
    PHASES = int(os.environ.get("BASSK_PHASES", "4"))  # 1=conv 2=+exchange 3=+gather 4=full
    NBLK, NMAX, NG = nblk, nblk * LANES, nblk * NCH
    nc = bacc.Bacc("TRN2", target_bir_lowering=False, debug=False,
                   num_devices=num_devices, dynamic_dma_scratch_size=32768)

    d_x = nc.dram_tensor("x_halo", [CIN, HROWS, W], BF16, kind="ExternalInput").ap()
    d_w1 = nc.dram_tensor("w1t", [12, 128, 128], BF16, kind="ExternalInput").ap()
    d_b1 = nc.dram_tensor("b1", [128, 2], F32, kind="ExternalInput").ap()
    d_w2 = nc.dram_tensor("w2t", [2, 128, 64], BF16, kind="ExternalInput").ap()
    d_b2 = nc.dram_tensor("b2", [64, 1], F32, kind="ExternalInput").ap()
    d_gidx = nc.dram_tensor("gidx", [NG, 128, NIDX // 16], I16, kind="ExternalInput").ap()
    d_wsb = nc.dram_tensor("wsb", [128, NBLK * SLOTS * 3], F32, kind="ExternalInput").ap()
    d_wpt = nc.dram_tensor("wpt", [KPAD, 512], BF16, kind="ExternalInput").ap()
    d_wft = nc.dram_tensor("wft", [512, 256], BF16, kind="ExternalInput").ap()
    d_bf = nc.dram_tensor("bfu", [128, 2], F32, kind="ExternalInput").ap()
    d_oft = nc.dram_tensor("oft", [2, 128, NMAX], F32, kind="ExternalOutput").ap()

    f_own = nc.dram_tensor("f_own", [FHALF, 64], BF16).ap()
    f_full = nc.dram_tensor("f_full", [FROWS, 64], BF16).ap()

    with tile.TileContext(nc) as tc:
        with (
            tc.tile_pool(name="persist", bufs=1) as pp,
            tc.tile_pool(name="gat", bufs=(4 if nblk == 2 else 3)) as gp_,
        ):
            t_id = pp.tile([128, 128], BF16)
            make_identity(nc, t_id[:])

            # ------------- phase 1: conv over own half -------------
            with (
                tc.tile_pool(name="convw", bufs=1) as cw,
                tc.tile_pool(name="convx", bufs=1) as cx,
                tc.tile_pool(name="convt", bufs=3) as ct,
                tc.tile_pool(name="psA", bufs=2, space="PSUM") as psA,
                tc.tile_pool(name="psB", bufs=2, space="PSUM") as psB,
                tc.tile_pool(name="psT", bufs=2, space="PSUM") as psT,
            ):
                t_w1 = cw.tile([128, 12 * 128], BF16)
                nc.sync.dma_start(out=t_w1[:].rearrange("k (j o) -> k j o", j=12),
                                  in_=d_w1.rearrange("j k o -> k j o"))
                t_b1 = cw.tile([128, 2], F32)
                nc.sync.dma_start(out=t_b1[:], in_=d_b1)
                t_w2 = cw.tile([128, 2 * 64], BF16)
                nc.sync.dma_start(out=t_w2[:].rearrange("k (c o) -> k c o", c=2),
                                  in_=d_w2.rearrange("c k o -> k c o"))
                t_b2 = cw.tile([64, 1], F32)
                nc.sync.dma_start(out=t_b2[:], in_=d_b2)

                # x2: [128, 130*258] top=ch(row r), bottom=ch(row r+1), cols zero-padded
                t_x2 = cx.tile([128, HROWS * WPAD], BF16)
                x2v = t_x2[:].rearrange("p (r c) -> p r c", r=HROWS)
                nc.vector.memset(x2v[:, :, 0:1], 0)
                nc.vector.memset(x2v[:, :, W + 1:W + 2], 0)
                nc.vector.memset(x2v[64:128, HROWS - 1:HROWS, :], 0)
                nc.sync.dma_start(out=x2v[0:64, :, 1:W + 1], in_=d_x)
                nc.sync.dma_start(out=x2v[64:128, 0:HROWS - 1, 1:W + 1],
                                  in_=d_x[:, 1:, :])

                def rhs_view(row0, dx):
                    off = t_x2[:].offset + row0 * WPAD + 1 + dx
                    return bass.AP(tensor=t_x2.tensor, offset=off,
                                   ap=[list(t_x2[:].ap[0]), [WPAD, 2], [1, W]])

                zeros64 = cw.tile([128, 64], BF16)
                nc.vector.memset(zeros64[:], 0)

                for t in range(64):
                    y0 = 2 * t
                    f1 = []
                    for m in range(2):
                        p1 = psA.tile([128, 512], F32, tag="p1")
                        for j in range(3):       # tap pairs ky=0,1
                            nc.tensor.matmul(
                                p1[:].rearrange("p (r c) -> p r c", r=2),
                                lhsT=t_w1[:, (m * 6 + j) * 128:(m * 6 + j + 1) * 128],
                                rhs=rhs_view(y0, j - 1),
                                start=(j == 0), stop=False)
                        for j in range(3):       # masked ky=2
                            nc.tensor.matmul(
                                p1[:].rearrange("p (r c) -> p r c", r=2),
                                lhsT=t_w1[:, (m * 6 + 3 + j) * 128:(m * 6 + 4 + j) * 128],
                                rhs=rhs_view(y0 + 1, j - 1),
                                start=False, stop=(j == 2))
                        t_f1 = ct.tile([128, 512], BF16, tag="f1")
                        nc.scalar.activation(out=t_f1[:], in_=p1[:], func=AF.Relu,
                                             bias=t_b1[:, m:m + 1])
                        f1.append(t_f1)
                    p2 = psB.tile([64, 512], F32, tag="p2")
                    for cch in range(2):
                        nc.tensor.matmul(p2[:], lhsT=t_w2[:, cch * 64:(cch + 1) * 64],
                                         rhs=f1[cch][:], start=(cch == 0), stop=(cch == 1))
                    t_f2 = ct.tile([64, 512], BF16, tag="f2")
                    nc.scalar.activation(out=t_f2[:], in_=p2[:], func=AF.Identity,
                                         bias=t_b2[:])
                    t_fs = ct.tile([128, 4 * 64], BF16, tag="fs")
                    for i in range(4):
                        ptr = psT.tile([128, 64], BF16, tag="ptr")
                        nc.tensor.transpose(out=ptr[:], in_=t_f2[:, i * 128:(i + 1) * 128],
                                            identity=t_id[0:64, 0:64])
                        nc.vector.tensor_copy(out=t_fs[:, i * 64:(i + 1) * 64], in_=ptr[:])
                    nc.sync.dma_start(
                        out=f_own[t * 512:(t + 1) * 512, :].rearrange(
                            "(i l) c -> l i c", i=4),
                        in_=t_fs[:].rearrange("l (i c) -> l i c", i=4))

                # zero f_full pad rows
                nc.sync.dma_start(
                    out=f_full[2 * FHALF:FROWS, :].rearrange("(i l) c -> l i c", i=1),
                    in_=zeros64[:].rearrange("l (i c) -> l i c", i=1))

            # ------------- exchange: pair AllGather (4 chunks) -------------
            # f_full layout is chunk-interleaved: [own_c0, peer_c0, own_c1, ...]
            # so each chunk's AllGather output is contiguous. Host remaps the
            # gather indices to match.
            CH = FHALF // NEX
            for ci in range(NEX if PHASES >= 2 else 0):
                nc.gpsimd.collective_compute(
                    "AllGather", mybir.AluOpType.bypass, replica_groups=pairs,
                    ins=[f_own[ci * CH:(ci + 1) * CH, :]],
                    outs=[f_full[2 * ci * CH:2 * (ci + 1) * CH, :]])

            # ------------- phase 2: gather + bilinear combine -------------
            NROWSV = (FROWS * 64 - 256) // 128 + 1   # 32831, > max idx 32767
            f_rows = bass.AP(tensor=f_full.tensor, offset=0,
                             ap=[[128, NROWSV], [1, 256]])
            with (
                tc.tile_pool(name="wsb", bufs=1) as wp_,
                tc.tile_pool(name="feat", bufs=1) as fp_,
                tc.tile_pool(name="comb", bufs=2) as cb_,
                tc.tile_pool(name="combh", bufs=1) as ch_,
            ):
                t_wsb = wp_.tile([128, NBLK * SLOTS * 3], F32)
                nc.sync.dma_start(out=t_wsb[:], in_=d_wsb)
                feat = [fp_.tile([128, KPAD], BF16, tag=f"feat{k}", name=f"feat{k}")
                        for k in range(NBLK)]
                for k in range(NBLK):
                    nc.vector.memset(feat[k][:], 0)

                for k in range(NBLK if PHASES >= 3 else 0):
                    for ci in range(NCH):
                        g = k * NCH + ci
                        t_idx = gp_.tile([128, NIDX // 16], I16, tag="idx")
                        nc.sync.dma_start(out=t_idx[:], in_=d_gidx[g])
                        t_g = gp_.tile([128, SC * 256], BF16, tag="g")
                        nc.gpsimd.dma_gather(
                            out_ap=t_g[:].rearrange("p (s e) -> p s e", s=SC),
                            in_ap=f_rows, idxs_ap=t_idx[:],
                            num_idxs=NIDX, num_idxs_reg=NIDX,
                            elem_size=256, elem_step=128,
                            single_packet=False)
                        # weight replication on ACT (keeps GpSimd free for desc-gen)
                        t_rep = cb_.tile([128, SC * 192], BF16, tag="rep")
                        col0 = (k * SLOTS + ci * SC) * 3
                        w_bc = bass.AP(
                            tensor=t_wsb.tensor, offset=t_wsb[:].offset + col0,
                            ap=[list(t_wsb[:].ap[0]), [3, SC], [1, 3], [0, 64]])
                        rep3 = t_rep[:].rearrange("p (s q c) -> p s q c", s=SC, q=3)
                        nc.scalar.activation(out=rep3, in_=w_bc, func=AF.Copy)
                        # in-place: g[:, :, 0:3, :] *= rep ; h1 = p0+p1 ; feat (+)= h1+p2
                        g4 = t_g[:].rearrange("p (s q c) -> p s q c", s=SC, q=4)
                        nc.vector.tensor_mul(out=g4[:, :, 0:3, :],
                                             in0=g4[:, :, 0:3, :], in1=rep3)
                        t_h1 = cb_.tile([128, SC * 64], BF16, tag="h1")
                        h1v = t_h1[:].rearrange("p (s c) -> p s c", s=SC)
                        nc.vector.tensor_add(out=h1v, in0=g4[:, :, 0, :],
                                             in1=g4[:, :, 1, :])
                        # accumulate into feat cols [ptbase*64, +SC*64)
                        ptbase = (ci % 3) * SC
                        fslice = feat[k][:, ptbase * 64:(ptbase + SC) * 64]
                        fv = fslice.rearrange("p (s c) -> p s c", s=SC)
                        if ci < 3:
                            nc.vector.tensor_add(out=fv, in0=h1v, in1=g4[:, :, 2, :])
                        else:
                            t_h = ch_.tile([128, SC * 64], BF16, tag="h")
                            hv = t_h[:].rearrange("p (s c) -> p s c", s=SC)
                            nc.vector.tensor_add(out=hv, in0=h1v, in1=g4[:, :, 2, :])
                            nc.vector.tensor_add(out=fv, in0=fv, in1=hv)

                # ------------- phase 3: poly + fuse matmuls -------------
                with (
                    tc.tile_pool(name="mm3", bufs=3) as m3,
                    tc.tile_pool(name="out3", bufs=2) as o3,
                    tc.tile_pool(name="psO", bufs=1, space="PSUM") as psO,
                    tc.tile_pool(name="psT3", bufs=3, space="PSUM") as psT3,
                    tc.tile_pool(name="psF", bufs=2, space="PSUM") as psF,
                ):
                    t_wf = wp_.tile([128, 4 * 256], BF16)
                    nc.sync.dma_start(out=t_wf[:].rearrange("k (i o) -> k i o", i=4),
                                      in_=d_wft.rearrange("(i k) o -> k i o", i=4))
                    t_bf = wp_.tile([128, 2], F32)
                    nc.sync.dma_start(out=t_bf[:], in_=d_bf)

                    p_off = [psO.tile([128, 512], F32, tag=f"off{k}", name=f"off{k}")
                             for k in range(NBLK)]
                    if PHASES < 4:
                        t_z = o3.tile([128, NMAX], F32, tag="z")
                        nc.vector.memset(t_z[:], 0)
                        for m in range(2):
                            nc.sync.dma_start(out=d_oft[m], in_=t_z[:])
                    for cc in range(KPAD // 128 if PHASES >= 4 else 0):
                        t_wp = m3.tile([128, 512], BF16, tag="wp")
                        nc.sync.dma_start(out=t_wp[:], in_=d_wpt[cc * 128:(cc + 1) * 128, :])
                        for k in range(NBLK):
                            ptr = psT3.tile([128, 128], BF16, tag="ptr3")
                            nc.tensor.transpose(
                                out=ptr[:], in_=feat[k][:, cc * 128:(cc + 1) * 128],
                                identity=t_id[:])
                            t_ft = m3.tile([128, 128], BF16, tag="ft")
                            nc.vector.tensor_copy(out=t_ft[:], in_=ptr[:])
                            nc.tensor.matmul(p_off[k][:], lhsT=t_ft[:], rhs=t_wp[:],
                                             start=(cc == 0), stop=(cc == KPAD // 128 - 1))

                    for k in range(NBLK if PHASES >= 4 else 0):
                        t_off = o3.tile([128, 512], BF16, tag="offsb")
                        nc.scalar.activation(out=t_off[:], in_=p_off[k][:], func=AF.Copy)
                        t_offT = o3.tile([128, 4 * 128], BF16, tag="offT")
                        for i in range(4):
                            ptr = psT3.tile([128, 128], BF16, tag="ptr3")
                            nc.tensor.transpose(out=ptr[:],
                                                in_=t_off[:, i * 128:(i + 1) * 128],
                                                identity=t_id[:])
                            nc.vector.tensor_copy(out=t_offT[:, i * 128:(i + 1) * 128],
                                                  in_=ptr[:])
                        for m in range(2):
                            p_f = psF.tile([128, 128], F32, tag="pf")
                            for i in range(4):
                                nc.tensor.matmul(
                                    p_f[:],
                                    lhsT=t_wf[:, i * 256 + m * 128:i * 256 + (m + 1) * 128],
                                    rhs=t_offT[:, i * 128:(i + 1) * 128],
                                    start=(i == 0), stop=(i == 3))
                            t_out = o3.tile([128, 128], F32, tag="out")
                            nc.scalar.activation(out=t_out[:], in_=p_f[:],
                                                 func=AF.Identity,
                                                 bias=t_bf[:, m:m + 1])
                            nc.sync.dma_start(out=d_oft[m, :, k * 128:(k + 1) * 128],
                                              in_=t_out[:])

    nc.compile()
    return nc


def _get_program(num_devices, pairs, nblk):
    key = (num_devices, tuple(map(tuple, pairs)), nblk)
    if key not in _PROG_CACHE:
        _PROG_CACHE[key] = _build_program(num_devices, pairs, nblk)
    return _PROG_CACHE[key]


def host_prep(cnn_feature, wh_pred, ct_ind, ct_img_idx, conv1_w, conv1_b,
              conv2_w, conv2_b, w_poly, w_fuse, b_fuse, n_cores=8):
    """All host-side math. Returns (in_maps, ids_per_core, out0, nblk)."""
    ct_ind = np.asarray(ct_ind).astype(np.int64)
    ct_img = np.asarray(ct_img_idx).astype(np.int64)
    N = ct_ind.shape[0]

    ct_x = np.clip(ct_ind % W, 0, W - 1).astype(np.int32)
    ct_y = np.clip(ct_ind // W, 0, H - 1).astype(np.int32)
    ct = np.stack([ct_x, ct_y], 1).astype(np.float32)            # [N,2]
    ct_off = wh_pred[ct_img, :, ct_y, ct_x].reshape(N, NPT, 2)   # [N,128,2]
    init_polys = ct_off * INIT_STRIDE + ct[:, None, :]
    out0 = (init_polys * DOWN).astype(np.float32)                # output[0]

    pts = np.concatenate([ct[:, None, :], init_polys], axis=1)   # [N,129,2]
    sx = pts[..., 0] - 0.5
    sy = pts[..., 1] - 0.5
    x0 = np.floor(sx).astype(np.int64)
    y0 = np.floor(sy).astype(np.int64)
    wx1 = (sx - x0).astype(np.float32); wx0 = 1.0 - wx1
    wy1 = (sy - y0).astype(np.float32); wy0 = 1.0 - wy1

    IDX = np.zeros((N, NP1, 2), np.int16)
    Wt = np.zeros((N, NP1, 2, 3), np.float32)
    p = (np.clip(x0, 0, W - 2) >> 1).astype(np.int64)
    x0v = (x0 >= 0) & (x0 <= W - 1)
    x1v = (x0 + 1 >= 0) & (x0 + 1 <= W - 1)
    for yc in range(2):
        yy = y0 + yc
        yval = (yy >= 0) & (yy <= H - 1)
        wy = (wy0 if yc == 0 else wy1) * yval
        ry = np.clip(yy, 0, H - 1)
        # f_full is chunk-interleaved by the pair exchange: pair index
        # qp within a half maps to (qp//CHP)*2*CHP + half*CHP + qp%CHP
        CHP = FHALF // NEX // 2                     # pairs per exchange chunk
        hh_r = ry // (H // 2)
        qp = (ry % (H // 2)) * (W // 2) + p
        IDX[:, :, yc] = ((qp // CHP) * 2 * CHP + hh_r * CHP
                         + (qp % CHP)).astype(np.int16)
        for pos in range(3):
            px = 2 * p + pos
            w = wy * (wx0 * ((px == x0) & x0v) + wx1 * ((px == x0 + 1) & x1v))
            Wt[:, :, yc, pos] = w

    # routing: image b -> cores 2b, 2b+1; sort by center position so each
    # gather slot's 128 descriptors hit clustered HBM addresses
    ids_per_core = []
    for c in range(n_cores):
        b, hh = c // 2, c % 2
        idb = np.where(ct_img == b)[0]
        ids = idb[hh::2]
        ids = ids[np.argsort(ct_y[ids] * W + ct_x[ids], kind="stable")]
        ids_per_core.append(ids)
    maxn = max(len(i) for i in ids_per_core)
    nblk = 2 if maxn <= 2 * LANES else 3
    NBLK, NMAX, NG = nblk, nblk * LANES, nblk * NCH
    assert maxn <= NMAX, f"max {maxn} instances on one core > {NMAX}"

    # static weights (shared across cores)
    w1t = np.zeros((12, 128, 128), np.float32)
    for m in range(2):
        for j in range(3):
            # pairs: k<64 -> ky=0, k>=64 -> ky=1
            w1t[m * 6 + j, 0:64, :] = conv1_w[m * 128:(m + 1) * 128, :, 0, j].T
            w1t[m * 6 + j, 64:128, :] = conv1_w[m * 128:(m + 1) * 128, :, 1, j].T
            # masked: k>=64 -> ky=2
            w1t[m * 6 + 3 + j, 64:128, :] = conv1_w[m * 128:(m + 1) * 128, :, 2, j].T
    w1t = w1t.astype(bf16)
    b1 = conv1_b.reshape(2, 128).T.copy().astype(np.float32)      # [128, 2]
    w2t = conv2_w[:, :, 0, 0].T.reshape(2, 128, 64).astype(bf16)  # [2,128,64]
    b2 = conv2_b.reshape(64, 1).astype(np.float32)
    wpt = np.zeros((KPAD, 512), np.float32)
    wpt[:KPOLY] = w_poly.reshape(512, 64, NP1).transpose(2, 1, 0).reshape(KPOLY, 512)
    wpt = wpt.astype(bf16)
    wft = w_fuse.T.copy().astype(bf16)                            # [512, 256]
    bfu = b_fuse.reshape(2, 128).T.copy().astype(np.float32)      # [128, 2]

    xf = cnn_feature.astype(bf16)
    in_maps = []
    for c in range(n_cores):
        b, hh = c // 2, c % 2
        ids = ids_per_core[c]
        # input halo [64, 130, 256]
        xh = np.zeros((CIN, HROWS, W), bf16)
        r0 = hh * 128 - 1
        lo, hi = max(r0, 0), min(r0 + HROWS, H)
        xh[:, lo - r0:hi - r0, :] = xf[b, :, lo:hi, :]
        # gather indices + weights, padded to NMAX lanes
        idx_c = np.zeros((NMAX, NP1, 2), np.int16)
        wt_c = np.zeros((NMAX, NP1, 2, 3), np.float32)
        idx_c[:len(ids)] = IDX[ids]
        wt_c[:len(ids)] = Wt[ids]
        gidx = np.zeros((NG, 128, NIDX // 16), np.int16)
        for k in range(NBLK):
            for ci in range(NCH):
                s0 = ci * SC
                yc, ptv = (s0 + np.arange(SC)) // NP1, (s0 + np.arange(SC)) % NP1
                # lin[j]: slot j//128 local, lane j%128
                lin = idx_c[k * 128:(k + 1) * 128, ptv, yc].T.reshape(-1)  # [SC*128]
                wrapped = lin.reshape(NIDX // 16, 16).T                    # [16, NIDX/16]
                gidx[k * NCH + ci] = np.tile(wrapped, (8, 1))
        # wsb cols: (k*258 + s)*3 + pos, s = yc*129 + pt
        wsb = wt_c.reshape(NBLK, 128, NP1, 2, 3).transpose(1, 0, 3, 2, 4).reshape(
            128, NBLK * SLOTS * 3).copy()
        in_maps.append({
            "x_halo": xh, "w1t": w1t, "b1": b1, "w2t": w2t, "b2": b2,
            "gidx": gidx, "wsb": wsb, "wpt": wpt, "wft": wft, "bfu": bfu,
        })
    return in_maps, ids_per_core, out0, nblk


def assemble(results, ids_per_core, out0):
    N = out0.shape[0]
    off2 = np.zeros((N, 256), np.float32)
    for c, ids in enumerate(ids_per_core):
        oft = results[c]["oft"]          # [2, 128, NMAX]
        n = len(ids)
        off2[ids, 0:128] = oft[0, :, :n].T
        off2[ids, 128:256] = oft[1, :, :n].T
    out1 = off2.reshape(N, NPT, 2) * (COARSE_STRIDE * DOWN) + out0
    return np.stack([out0, out1]).astype(np.float32)


def kernel(**inputs):
    global LAST_EXEC_NS, LAST_RESULT
    inputs = {k: np.asarray(v) for k, v in inputs.items()}
    in_maps, ids_per_core, out0, nblk = host_prep(**inputs, n_cores=8)
    nc = _get_program(8, [[0, 1], [2, 3], [4, 5], [6, 7]], nblk)
    res = run_bass_kernel_spmd(nc, in_maps, list(range(8)), trace=TRACE)
    LAST_EXEC_NS = res.exec_time_ns
    LAST_RESULT = res
    return assemble(res.results, ids_per_core, out0)



# revision 2
# speedup vs baseline: 1.5495x; 1.5495x over previous
"""nn_Decode (CenterNet-style polygon decode) on 8 Trainium2 NeuronCores.

Strategy (data-parallel over batch, instance-routed gather):
  host:   all index math: wh_pred center gather (host-known indices), init_polys,
          bilinear corner indices + weights, instance->core routing by image,
          weight layout transforms, bf16 casts.
  device: per core (c = 2*b + h) = (image b, half h):
          conv3x3(64->256)+ReLU+conv1x1(256->64) on its half-image via im2col
          matmuls (bf16, row-pair K-packing), f written pixel-major bf16 to DRAM,
          pair AllGather -> full-image f, dma_gather (int16 idx, 512B rows of
          4px x 64ch), DVE bilinear combine with ACT/GPSIMD weight replication,
          PE transposes -> poly matmul (K=8320) -> fuse matmul -> off2^T out.
  host:   out[0] = init*4 (exact), out[1] = off2*16 + out[0].
"""
import os
import numpy as np
import ml_dtypes

import concourse.bass as bass
import concourse.mybir as mybir
import concourse.tile as tile
from concourse import bacc
from concourse.bass_utils import run_bass_kernel_spmd
from concourse.masks import make_identity

BF16, F32, I16 = mybir.dt.bfloat16, mybir.dt.float32, mybir.dt.int16
AF = mybir.ActivationFunctionType
bf16 = ml_dtypes.bfloat16

# problem constants (hardcoded per spec)
B, CIN, H, W = 4, 64, 256, 256
C1 = 256                 # conv1 out channels
NPT, NP1, NINST = 128, 129, 2000
INIT_STRIDE, COARSE_STRIDE, DOWN = 10.0, 4.0, 4.0

LANES = 128                       # instances per block
SLOTS = 2 * NP1                   # 258 gather slots per block, yc-major: s = yc*129+pt
SC = 43                           # slots per gather chunk
NCH = SLOTS // SC                 # 6 chunks per block (chunks 0-2: yc=0, 3-5: yc=1)
NIDX = SC * LANES                 # 5504 indices per gather
KPOLY, KPAD = NP1 * 64, 65 * 128  # 8256, 8320
HROWS = 130                       # input halo rows per half
WPAD = W + 2                      # 258, zero-padded row width
FHALF = (H // 2) * W              # 32768 px per half
FROWS = 2 * FHALF + 128           # f_full rows incl. pad
NEX = 8                           # exchange chunks (f_full chunk-interleaved)

_PROG_CACHE = {}
TRACE = False          # test harness sets True to capture NTFF profile
LAST_EXEC_NS = None
LAST_RESULT = None


def _build_program(num_devices, pairs, nblk):
    PHASES = int(os.environ.get("BASSK_PHASES", "4"))  # 1=conv 2=+exchange 3=+gather 4=full
    NBLK, NMAX, NG = nblk, nblk * LANES, nblk * NCH
    nc = bacc.Bacc("TRN2", target_bir_lowering=False, debug=False,
                   num_devices=num_devices, dynamic_dma_scratch_size=32768)

    d_x = nc.dram_tensor("x_halo", [CIN, HROWS, W], BF16, kind="ExternalInput").ap()
    d_w1 = nc.dram_tensor("w1t", [12, 128, 128], BF16, kind="ExternalInput").ap()
    d_b1 = nc.dram_tensor("b1", [128, 2], F32, kind="ExternalInput").ap()
    d_w2 = nc.dram_tensor("w2t", [2, 128, 64], BF16, kind="ExternalInput").ap()
    d_b2 = nc.dram_tensor("b2", [64, 1], F32, kind="ExternalInput").ap()
    d_gidx = nc.dram_tensor("gidx", [NG, 128, NIDX // 16], I16, kind="ExternalInput").ap()
    d_wsb = nc.dram_tensor("wsb", [128, NBLK * SLOTS * 3], F32, kind="ExternalInput").ap()
    d_wpt = nc.dram_tensor("wpt", [KPAD, 512], BF16, kind="ExternalInput").ap()
    d_wft = nc.dram_tensor("wft", [512, 256], BF16, kind="ExternalInput").ap()
    d_bf = nc.dram_tensor("bfu", [128, 2], F32, kind="ExternalInput").ap()
    d_oft = nc.dram_tensor("oft", [2, 128, NMAX], F32, kind="ExternalOutput").ap()

    f_own = nc.dram_tensor("f_own", [FHALF, 64], BF16).ap()
    f_full = nc.dram_tensor("f_full", [FROWS, 64], BF16).ap()

    with tile.TileContext(nc) as tc:
        with (
            tc.tile_pool(name="persist", bufs=1) as pp,
            tc.tile_pool(name="gat", bufs=(4 if nblk == 2 else 3)) as gp_,
        ):
            t_id = pp.tile([128, 128], BF16)
            make_identity(nc, t_id[:])

            # ------------- phase 1: conv over own half -------------
            with (
                tc.tile_pool(name="convw", bufs=1) as cw,
                tc.tile_pool(name="convx", bufs=1) as cx,
                tc.tile_pool(name="convt", bufs=3) as ct,
                tc.tile_pool(name="psA", bufs=2, space="PSUM") as psA,
                tc.tile_pool(name="psB", bufs=2, space="PSUM") as psB,
                tc.tile_pool(name="psT", bufs=2, space="PSUM") as psT,
            ):
                t_w1 = cw.tile([128, 12 * 128], BF16)
                nc.sync.dma_start(out=t_w1[:].rearrange("k (j o) -> k j o", j=12),
                                  in_=d_w1.rearrange("j k o -> k j o"))
                t_b1 = cw.tile([128, 2], F32)
                nc.sync.dma_start(out=t_b1[:], in_=d_b1)
                t_w2 = cw.tile([128, 2 * 64], BF16)
                nc.sync.dma_start(out=t_w2[:].rearrange("k (c o) -> k c o", c=2),
                                  in_=d_w2.rearrange("c k o -> k c o"))
                t_b2 = cw.tile([64, 1], F32)
                nc.sync.dma_start(out=t_b2[:], in_=d_b2)

                # x2: [128, 130*258] top=ch(row r), bottom=ch(row r+1), cols zero-padded
                t_x2 = cx.tile([128, HROWS * WPAD], BF16)
                x2v = t_x2[:].rearrange("p (r c) -> p r c", r=HROWS)
                nc.vector.memset(x2v[:, :, 0:1], 0)
                nc.vector.memset(x2v[:, :, W + 1:W + 2], 0)
                nc.vector.memset(x2v[64:128, HROWS - 1:HROWS, :], 0)
                nc.sync.dma_start(out=x2v[0:64, :, 1:W + 1], in_=d_x)
                nc.sync.dma_start(out=x2v[64:128, 0:HROWS - 1, 1:W + 1],
                                  in_=d_x[:, 1:, :])

                def rhs_view(row0, dx):
                    off = t_x2[:].offset + row0 * WPAD + 1 + dx
                    return bass.AP(tensor=t_x2.tensor, offset=off,
                                   ap=[list(t_x2[:].ap[0]), [WPAD, 2], [1, W]])

                zeros64 = cw.tile([128, 64], BF16)
                nc.vector.memset(zeros64[:], 0)

                for t in range(64):
                    y0 = 2 * t
                    f1 = []
                    for m in range(2):
                        p1 = psA.tile([128, 512], F32, tag="p1")
                        for j in range(3):       # tap pairs ky=0,1
                            nc.tensor.matmul(
                                p1[:].rearrange("p (r c) -> p r c", r=2),
                                lhsT=t_w1[:, (m * 6 + j) * 128:(m * 6 + j + 1) * 128],
                                rhs=rhs_view(y0, j - 1),
                                start=(j == 0), stop=False)
                        for j in range(3):       # masked ky=2
                            nc.tensor.matmul(
                                p1[:].rearrange("p (r c) -> p r c", r=2),
                                lhsT=t_w1[:, (m * 6 + 3 + j) * 128:(m * 6 + 4 + j) * 128],
                                rhs=rhs_view(y0 + 1, j - 1),
                                start=False, stop=(j == 2))
                        t_f1 = ct.tile([128, 512], BF16, tag="f1")
                        nc.scalar.activation(out=t_f1[:], in_=p1[:], func=AF.Relu,
                                             bias=t_b1[:, m:m + 1])
                        f1.append(t_f1)
                    p2 = psB.tile([64, 512], F32, tag="p2")
                    for cch in range(2):
                        nc.tensor.matmul(p2[:], lhsT=t_w2[:, cch * 64:(cch + 1) * 64],
                                         rhs=f1[cch][:], start=(cch == 0), stop=(cch == 1))
                    t_f2 = ct.tile([64, 512], BF16, tag="f2")
                    nc.scalar.activation(out=t_f2[:], in_=p2[:], func=AF.Identity,
                                         bias=t_b2[:])
                    t_fs = ct.tile([128, 4 * 64], BF16, tag="fs")
                    for i in range(4):
                        ptr = psT.tile([128, 64], BF16, tag="ptr")
                        nc.tensor.transpose(out=ptr[:], in_=t_f2[:, i * 128:(i + 1) * 128],
                                            identity=t_id[0:64, 0:64])
                        nc.vector.tensor_copy(out=t_fs[:, i * 64:(i + 1) * 64], in_=ptr[:])
                    nc.sync.dma_start(
                        out=f_own[t * 512:(t + 1) * 512, :].rearrange(
                            "(i l) c -> l i c", i=4),
                        in_=t_fs[:].rearrange("l (i c) -> l i c", i=4))

                # zero f_full pad rows
                nc.sync.dma_start(
                    out=f_full[2 * FHALF:FROWS, :].rearrange("(i l) c -> l i c", i=1),
                    in_=zeros64[:].rearrange("l (i c) -> l i c", i=1))

            # ------------- exchange: pair AllGather (4 chunks) -------------
            # f_full layout is chunk-interleaved: [own_c0, peer_c0, own_c1, ...]
            # so each chunk's AllGather output is contiguous. Host remaps the
            # gather indices to match.
            CH = FHALF // NEX
            for ci in range(NEX if PHASES >= 2 else 0):
                nc.gpsimd.collective_compute(
                    "AllGather", mybir.AluOpType.bypass, replica_groups=pairs,
                    ins=[f_own[ci * CH:(ci + 1) * CH, :]],
                    outs=[f_full[2 * ci * CH:2 * (ci + 1) * CH, :]])

            # ------------- phase 2: gather + bilinear combine -------------
            NROWSV = (FROWS * 64 - 256) // 128 + 1   # 32831, > max idx 32767
            f_rows = bass.AP(tensor=f_full.tensor, offset=0,
                             ap=[[128, NROWSV], [1, 256]])
            with (
                tc.tile_pool(name="wsb", bufs=1) as wp_,
                tc.tile_pool(name="feat", bufs=1) as fp_,
                tc.tile_pool(name="comb", bufs=2) as cb_,
                tc.tile_pool(name="combh", bufs=1) as ch_,
            ):
                t_wsb = wp_.tile([128, NBLK * SLOTS * 3], F32)
                nc.sync.dma_start(out=t_wsb[:], in_=d_wsb)
                feat = [fp_.tile([128, KPAD], BF16, tag=f"feat{k}", name=f"feat{k}")
                        for k in range(NBLK)]
                for k in range(NBLK):
                    nc.vector.memset(feat[k][:], 0)

                for k in range(NBLK if PHASES >= 3 else 0):
                    for ci in range(NCH):
                        g = k * NCH + ci
                        t_idx = gp_.tile([128, NIDX // 16], I16, tag="idx")
                        nc.sync.dma_start(out=t_idx[:], in_=d_gidx[g])
                        t_g = gp_.tile([128, SC * 256], BF16, tag="g")
                        nc.gpsimd.dma_gather(
                            out_ap=t_g[:].rearrange("p (s e) -> p s e", s=SC),
                            in_ap=f_rows, idxs_ap=t_idx[:],
                            num_idxs=NIDX, num_idxs_reg=NIDX,
                            elem_size=256, elem_step=128,
                            single_packet=False)
                        # weight replication on ACT (keeps GpSimd free for desc-gen)
                        t_rep = cb_.tile([128, SC * 192], BF16, tag="rep")
                        col0 = (k * SLOTS + ci * SC) * 3
                        w_bc = bass.AP(
                            tensor=t_wsb.tensor, offset=t_wsb[:].offset + col0,
                            ap=[list(t_wsb[:].ap[0]), [3, SC], [1, 3], [0, 64]])
                        rep3 = t_rep[:].rearrange("p (s q c) -> p s q c", s=SC, q=3)
                        nc.scalar.activation(out=rep3, in_=w_bc, func=AF.Copy)
                        # in-place: g[:, :, 0:3, :] *= rep ; h1 = p0+p1 ; feat (+)= h1+p2
                        g4 = t_g[:].rearrange("p (s q c) -> p s q c", s=SC, q=4)
                        nc.vector.tensor_mul(out=g4[:, :, 0:3, :],
                                             in0=g4[:, :, 0:3, :], in1=rep3)
                        t_h1 = cb_.tile([128, SC * 64], BF16, tag="h1")
                        h1v = t_h1[:].rearrange("p (s c) -> p s c", s=SC)
                        nc.vector.tensor_add(out=h1v, in0=g4[:, :, 0, :],
                                             in1=g4[:, :, 1, :])
                        # accumulate into feat cols [ptbase*64, +SC*64)
                        ptbase = (ci % 3) * SC
                        fslice = feat[k][:, ptbase * 64:(ptbase + SC) * 64]
                        fv = fslice.rearrange("p (s c) -> p s c", s=SC)
                        if ci < 3:
                            nc.vector.tensor_add(out=fv, in0=h1v, in1=g4[:, :, 2, :])
                        else:
                            t_h = ch_.tile([128, SC * 64], BF16, tag="h")
                            hv = t_h[:].rearrange("p (s c) -> p s c", s=SC)
                            nc.vector.tensor_add(out=hv, in0=h1v, in1=g4[:, :, 2, :])
                            nc.vector.tensor_add(out=fv, in0=fv, in1=hv)

                # ------------- phase 3: poly + fuse matmuls -------------
                with (
                    tc.tile_pool(name="mm3", bufs=3) as m3,
                    tc.tile_pool(name="out3", bufs=2) as o3,
                    tc.tile_pool(name="psO", bufs=1, space="PSUM") as psO,
                    tc.tile_pool(name="psT3", bufs=3, space="PSUM") as psT3,
                    tc.tile_pool(name="psF", bufs=2, space="PSUM") as psF,
                ):
                    t_wf = wp_.tile([128, 4 * 256], BF16)
                    nc.sync.dma_start(out=t_wf[:].rearrange("k (i o) -> k i o", i=4),
                                      in_=d_wft.rearrange("(i k) o -> k i o", i=4))
                    t_bf = wp_.tile([128, 2], F32)
                    nc.sync.dma_start(out=t_bf[:], in_=d_bf)

                    p_off = [psO.tile([128, 512], F32, tag=f"off{k}", name=f"off{k}")
                             for k in range(NBLK)]
                    if PHASES < 4:
                        t_z = o3.tile([128, NMAX], F32, tag="z")
                        nc.vector.memset(t_z[:], 0)
                        for m in range(2):
                            nc.sync.dma_start(out=d_oft[m], in_=t_z[:])
                    for cc in range(KPAD // 128 if PHASES >= 4 else 0):
                        t_wp = m3.tile([128, 512], BF16, tag="wp")
                        nc.sync.dma_start(out=t_wp[:], in_=d_wpt[cc * 128:(cc + 1) * 128, :])
                        for k in range(NBLK):
                            ptr = psT3.tile([128, 128], BF16, tag="ptr3")
                            nc.tensor.transpose(
                                out=ptr[:], in_=feat[k][:, cc * 128:(cc + 1) * 128],
                                identity=t_id[:])
                            t_ft = m3.tile([128, 128], BF16, tag="ft")
                            nc.vector.tensor_copy(out=t_ft[:], in_=ptr[:])
                            nc.tensor.matmul(p_off[k][:], lhsT=t_ft[:], rhs=t_wp[:],
                                             start=(cc == 0), stop=(cc == KPAD // 128 - 1))

                    for k in range(NBLK if PHASES >= 4 else 0):
                        t_off = o3.tile([128, 512], BF16, tag="offsb")
                        nc.scalar.activation(out=t_off[:], in_=p_off[k][:], func=AF.Copy)
                        t_offT = o3.tile([128, 4 * 128], BF16, tag="offT")
                        for i in range(4):
                            ptr = psT3.tile([128, 128], BF16, tag="ptr3")
                            nc.tensor.transpose(out=ptr[:],
                                                in_=t_off[:, i * 128:(i + 1) * 128],
                                                identity=t_id[:])
                            nc.vector.tensor_copy(out=t_offT[:, i * 128:(i + 1) * 128],
                                                  in_=ptr[:])
                        for m in range(2):
                            p_f = psF.tile([128, 128], F32, tag="pf")
                            for i in range(4):
                                nc.tensor.matmul(
                                    p_f[:],
                                    lhsT=t_wf[:, i * 256 + m * 128:i * 256 + (m + 1) * 128],
                                    rhs=t_offT[:, i * 128:(i + 1) * 128],
                                    start=(i == 0), stop=(i == 3))
                            t_out = o3.tile([128, 128], F32, tag="out")
                            nc.scalar.activation(out=t_out[:], in_=p_f[:],
                                                 func=AF.Identity,
                                                 bias=t_bf[:, m:m + 1])
                            nc.sync.dma_start(out=d_oft[m, :, k * 128:(k + 1) * 128],
                                              in_=t_out[:])

    nc.compile()
    return nc


def _get_program(num_devices, pairs, nblk):
    key = (num_devices, tuple(map(tuple, pairs)), nblk)
    if key not in _PROG_CACHE:
        _PROG_CACHE[key] = _build_program(num_devices, pairs, nblk)
    return _PROG_CACHE[key]


def host_prep(cnn_feature, wh_pred, ct_ind, ct_img_idx, conv1_w, conv1_b,
              conv2_w, conv2_b, w_poly, w_fuse, b_fuse, n_cores=8):
    """All host-side math. Returns (in_maps, ids_per_core, out0, nblk)."""
    ct_ind = np.asarray(ct_ind).astype(np.int64)
    ct_img = np.asarray(ct_img_idx).astype(np.int64)
    N = ct_ind.shape[0]

    ct_x = np.clip(ct_ind % W, 0, W - 1).astype(np.int32)
    ct_y = np.clip(ct_ind // W, 0, H - 1).astype(np.int32)
    ct = np.stack([ct_x, ct_y], 1).astype(np.float32)            # [N,2]
    ct_off = wh_pred[ct_img, :, ct_y, ct_x].reshape(N, NPT, 2)   # [N,128,2]
    init_polys = ct_off * INIT_STRIDE + ct[:, None, :]
    out0 = (init_polys * DOWN).astype(np.float32)                # output[0]

    pts = np.concatenate([ct[:, None, :], init_polys], axis=1)   # [N,129,2]
    sx = pts[..., 0] - 0.5
    sy = pts[..., 1] - 0.5
    x0 = np.floor(sx).astype(np.int64)
    y0 = np.floor(sy).astype(np.int64)
    wx1 = (sx - x0).astype(np.float32); wx0 = 1.0 - wx1
    wy1 = (sy - y0).astype(np.float32); wy0 = 1.0 - wy1

    IDX = np.zeros((N, NP1, 2), np.int16)
    Wt = np.zeros((N, NP1, 2, 3), np.float32)
    p = (np.clip(x0, 0, W - 2) >> 1).astype(np.int64)
    x0v = (x0 >= 0) & (x0 <= W - 1)
    x1v = (x0 + 1 >= 0) & (x0 + 1 <= W - 1)
    for yc in range(2):
        yy = y0 + yc
        yval = (yy >= 0) & (yy <= H - 1)
        wy = (wy0 if yc == 0 else wy1) * yval
        ry = np.clip(yy, 0, H - 1)
        # f_full is chunk-interleaved by the pair exchange: pair index
        # qp within a half maps to (qp//CHP)*2*CHP + half*CHP + qp%CHP
        CHP = FHALF // NEX // 2                     # pairs per exchange chunk
        hh_r = ry // (H // 2)
        qp = (ry % (H // 2)) * (W // 2) + p
        IDX[:, :, yc] = ((qp // CHP) * 2 * CHP + hh_r * CHP
                         + (qp % CHP)).astype(np.int16)
        for pos in range(3):
            px = 2 * p + pos
            w = wy * (wx0 * ((px == x0) & x0v) + wx1 * ((px == x0 + 1) & x1v))
            Wt[:, :, yc, pos] = w

    # routing: image b -> cores 2b, 2b+1; sort by center position so each
    # gather slot's 128 descriptors hit clustered HBM addresses
    ids_per_core = []
    for c in range(n_cores):
        b, hh = c // 2, c % 2
        idb = np.where(ct_img == b)[0]
        ids = idb[hh::2]
        ids = ids[np.argsort(ct_y[ids] * W + ct_x[ids], kind="stable")]
        ids_per_core.append(ids)
    maxn = max(len(i) for i in ids_per_core)
    nblk = 2 if maxn <= 2 * LANES else 3
    NBLK, NMAX, NG = nblk, nblk * LANES, nblk * NCH
    assert maxn <= NMAX, f"max {maxn} instances on one core > {NMAX}"

    # static weights (shared across cores)
    w1t = np.zeros((12, 128, 128), np.float32)
    for m in range(2):
        for j in range(3):
            # pairs: k<64 -> ky=0, k>=64 -> ky=1
            w1t[m * 6 + j, 0:64, :] = conv1_w[m * 128:(m + 1) * 128, :, 0, j].T
            w1t[m * 6 + j, 64:128, :] = conv1_w[m * 128:(m + 1) * 128, :, 1, j].T
            # masked: k>=64 -> ky=2
            w1t[m * 6 + 3 + j, 64:128, :] = conv1_w[m * 128:(m + 1) * 128, :, 2, j].T
    w1t = w1t.astype(bf16)
    b1 = conv1_b.reshape(2, 128).T.copy().astype(np.float32)      # [128, 2]
    w2t = conv2_w[:, :, 0, 0].T.reshape(2, 128, 64).astype(bf16)  # [2,128,64]
    b2 = conv2_b.reshape(64, 1).astype(np.float32)
    wpt = np.zeros((KPAD, 512), np.float32)
    wpt[:KPOLY] = w_poly.reshape(512, 64, NP1).transpose(2, 1, 0).reshape(KPOLY, 512)
    wpt = wpt.astype(bf16)
    wft = w_fuse.T.copy().astype(bf16)                            # [512, 256]
    bfu = b_fuse.reshape(2, 128).T.copy().astype(np.float32)      # [128, 2]

    xf = cnn_feature.astype(bf16)
    in_maps = []
    for c in range(n_cores):
        b, hh = c // 2, c % 2
        ids = ids_per_core[c]
        # input halo [64, 130, 256]
        xh = np.zeros((CIN, HROWS, W), bf16)
        r0 = hh * 128 - 1
        lo, hi = max(r0, 0), min(r0 + HROWS, H)
        xh[:, lo - r0:hi - r0, :] = xf[b, :, lo:hi, :]
        # gather indices + weights, padded to NMAX lanes. Pad lanes carry
        # zero weights but real (spread) indices: constant-index padding
        # makes every DMA engine hammer the same 512B row, serializing the
        # gather ~8x.
        spread = (np.arange(NMAX * NP1 * 2, dtype=np.int64) * 9973) % 32000
        idx_c = spread.reshape(NMAX, NP1, 2).astype(np.int16)
        wt_c = np.zeros((NMAX, NP1, 2, 3), np.float32)
        idx_c[:len(ids)] = IDX[ids]
        wt_c[:len(ids)] = Wt[ids]
        gidx = np.zeros((NG, 128, NIDX // 16), np.int16)
        for k in range(NBLK):
            for ci in range(NCH):
                s0 = ci * SC
                yc, ptv = (s0 + np.arange(SC)) // NP1, (s0 + np.arange(SC)) % NP1
                # lin[j]: slot j//128 local, lane j%128
                lin = idx_c[k * 128:(k + 1) * 128, ptv, yc].T.reshape(-1)  # [SC*128]
                wrapped = lin.reshape(NIDX // 16, 16).T                    # [16, NIDX/16]
                gidx[k * NCH + ci] = np.tile(wrapped, (8, 1))
        # wsb cols: (k*258 + s)*3 + pos, s = yc*129 + pt
        wsb = wt_c.reshape(NBLK, 128, NP1, 2, 3).transpose(1, 0, 3, 2, 4).reshape(
            128, NBLK * SLOTS * 3).copy()
        in_maps.append({
            "x_halo": xh, "w1t": w1t, "b1": b1, "w2t": w2t, "b2": b2,
            "gidx": gidx, "wsb": wsb, "wpt": wpt, "wft": wft, "bfu": bfu,
        })
    return in_maps, ids_per_core, out0, nblk


def assemble(results, ids_per_core, out0):
    N = out0.shape[0]
    off2 = np.zeros((N, 256), np.float32)
    for c, ids in enumerate(ids_per_core):
        oft = results[c]["oft"]          # [2, 128, NMAX]
        n = len(ids)
        off2[ids, 0:128] = oft[0, :, :n].T
        off2[ids, 128:256] = oft[1, :, :n].T
    out1 = off2.reshape(N, NPT, 2) * (COARSE_STRIDE * DOWN) + out0
    return np.stack([out0, out1]).astype(np.float32)


def kernel(**inputs):
    global LAST_EXEC_NS, LAST_RESULT
    inputs = {k: np.asarray(v) for k, v in inputs.items()}
    in_maps, ids_per_core, out0, nblk = host_prep(**inputs, n_cores=8)
    nc = _get_program(8, [[0, 1], [2, 3], [4, 5], [6, 7]], nblk)
    res = run_bass_kernel_spmd(nc, in_maps, list(range(8)), trace=TRACE)
    LAST_EXEC_NS = res.exec_time_ns
    LAST_RESULT = res
    return assemble(res.results, ids_per_core, out0)

